# revision 11
# baseline (speedup 1.0000x reference)
"""Trainium2 Bass kernel for nn_MultiHeadAttention (B=2, S=2048, E=1024, H=16).

Sharding (8 cores): core c handles batch b = c//4 and the 4 heads
[4*(c%4), 4*(c%4)+4) of that batch. Per core:
  1. QKV projection from the (transposed, bf16) input slab:
     Q^T, K^T in [d, s] layout (one head-pair per 128-partition tile),
     V in natural [s, d] layout with an extra all-ones column per head
     (the ones column makes the A@V matmul also produce the softmax
     denominator row).
  2. Flash-style causal attention per head pair over 512-query chunks:
     scores computed transposed S^T[k, q] one 128-key tile at a time
     (both heads of a pair into one 2-bank PSUM tile), exp on the Scalar
     engine (no max-subtraction: |scores| <= ~3 for these inputs), causal
     masking via an upper-triangular multiplicative mask on the diagonal
     block, A@V accumulated in PSUM; normalization uses a DVE reciprocal
     of the denominator row and a rank-1 PE matmul to broadcast it across
     partitions.
  3. Per head-pair, ctx^T is exchanged with one 8-core AllToAll: each core
     sends its pair ctx for s-quarter j to shard slots j and j+4, so every
     core receives the full-head ctx^T for its own s-quarter; rows from the
     other batch's cores are killed by zeros in the host-permuted stacked
     w_out. The pair-0 AllToAll overlaps pair-1 compute.
  4. Output projection (full E contraction over the 16 received chunks)
     produces out^T [1024, 512] for this core's s-quarter.
Host gathers the 8 [1024, 512] fp32 slabs into the [2, 2048, 1024] output.

All matmuls run in bf16 (fp32 PSUM accumulation); softmax statistics stay
fp32 except the broadcast reciprocal row (bf16).
"""
import sys

if '/opt/trn_rl_repo' not in sys.path:
    sys.path.insert(0, '/opt/trn_rl_repo')

from contextlib import ExitStack

import numpy as np
import ml_dtypes

import concourse.bass as bass
import concourse.bacc as bacc
import concourse.tile as tile
from concourse import mybir

BF16 = mybir.dt.bfloat16
F32 = mybir.dt.float32
EXP = mybir.ActivationFunctionType.Exp

B, S, E = 2, 2048, 1024
H, D = 16, 64
HPC = 4              # heads per core
N_CORES = 8
QC = 512             # query chunk
NQC = S // QC        # 4
NKT = S // 128       # 16 key tiles
SCALE = 1.0 / np.sqrt(D)


def build_nc(do_qkv=True, do_attn=True, do_cc=True, do_proj=True, reps=1):
    nc = bacc.Bacc("TRN2", target_bir_lowering=False, debug=False,
                   num_devices=N_CORES)

    xT = nc.dram_tensor("xT", [E, S], BF16, kind="ExternalInput")
    wqkT = nc.dram_tensor("wqkT", [E, 512], BF16, kind="ExternalInput")
    wvT = nc.dram_tensor("wvT", [E, 256], BF16, kind="ExternalInput")
    woutT = nc.dram_tensor("woutT", [2 * E, E], BF16, kind="ExternalInput")
    bqk = nc.dram_tensor("bqk", [1, 512], BF16, kind="ExternalInput")
    bv = nc.dram_tensor("bv", [1, 256], BF16, kind="ExternalInput")
    bout = nc.dram_tensor("bout", [1, E], BF16, kind="ExternalInput")
    out = nc.dram_tensor("out", [E, QC], F32, kind="ExternalOutput")

    cc_in = [nc.dram_tensor(f"cc_in{p}", [1024, QC], BF16) for p in range(2)]
    cc_out = [nc.dram_tensor(f"cc_out{p}", [1024, QC], BF16)
              for p in range(2)]

    tri_np = np.triu(np.ones((128, 128), np.float32)).astype(ml_dtypes.bfloat16)
    tri_dram = nc.inline_tensor(tri_np, name="tri_const")
    ones_np = np.ones((1, 512), ml_dtypes.bfloat16)
    ones_dram = nc.inline_tensor(ones_np, name="ones_const")

    with tile.TileContext(nc) as tc, ExitStack() as ctx:
        cp = ctx.enter_context(tc.tile_pool(name="const", bufs=1))
        wp = ctx.enter_context(tc.tile_pool(name="work", bufs=6))
        np2 = ctx.enter_context(tc.tile_pool(name="norm", bufs=2))
        ps = ctx.enter_context(tc.tile_pool(name="ps", bufs=2, space="PSUM"))
        sp2 = ctx.enter_context(tc.tile_pool(name="sp2", bufs=2, space="PSUM"))
        ctxp = ctx.enter_context(tc.tile_pool(name="ctxp", bufs=2, space="PSUM"))

        # ---- constant / input loads -------------------------------------
        xT_sb = cp.tile([128, 8 * S], BF16, tag="xT")
        nc.sync.dma_start(xT_sb[:], xT.ap().rearrange("(n p) m -> p n m", p=128))
        wqk_sb = cp.tile([128, 8 * 512], BF16, tag="wqk")
        nc.sync.dma_start(wqk_sb[:], wqkT.ap().rearrange("(n p) m -> p n m", p=128))
        wv_sb = cp.tile([128, 8 * 256], BF16, tag="wv")
        nc.sync.dma_start(wv_sb[:], wvT.ap().rearrange("(n p) m -> p n m", p=128))
        wout_sb = cp.tile([128, 16 * E], BF16, tag="wout")
        nc.sync.dma_start(wout_sb[:], woutT.ap().rearrange("(n p) m -> p n m", p=128))
        bqk_sb = cp.tile([1, 512], BF16, tag="bqk")
        nc.sync.dma_start(bqk_sb[:], bqk.ap())
        bv_sb = cp.tile([1, 256], BF16, tag="bv")
        nc.sync.dma_start(bv_sb[:], bv.ap())
        bout_sb = cp.tile([1, E], BF16, tag="bout")
        nc.sync.dma_start(bout_sb[:], bout.ap())
        tri_sb = cp.tile([128, 128], BF16, tag="tri")
        nc.sync.dma_start(tri_sb[:], tri_dram.ap())
        ones_sb = cp.tile([1, 512], BF16, tag="ones")
        nc.sync.dma_start(ones_sb[:], ones_dram.ap())

        # V slab: 16 s-tiles x 4 heads x (64 V cols + 1 ones col)
        v_sb = cp.tile([128, NKT * 260], BF16, tag="v")
        for t in range(NKT):
            blk = v_sb[:, 260 * t:260 * (t + 1)].rearrange(
                "p (h c) -> p h c", c=65)
            nc.vector.memset(blk[:, :, 64:65], 1.0)

        for _rep in range(reps):
            # ---- QKV projection helpers ---------------------------------
            qk_sb = {}

            def emit_qk(name, m):
                dst = cp.tile([128, S], BF16, tag=f"qk_{name}", name=f"qk_{name}")
                qk_sb[name] = dst
                for n in range(4):
                    acc = ps.tile([128, 512], F32, tag="ps", name=f"qkacc_{name}_{n}")
                    for k in range(8):
                        nc.tensor.matmul(
                            acc[:],
                            lhsT=wqk_sb[:, 512 * k + 128 * m: 512 * k + 128 * (m + 1)],
                            rhs=xT_sb[:, S * k + 512 * n: S * k + 512 * (n + 1)],
                            start=(k == 0), stop=False)
                    nc.tensor.matmul(
                        acc[:], lhsT=bqk_sb[0:1, 128 * m:128 * (m + 1)],
                        rhs=ones_sb[0:1, 0:512], start=False, stop=True)
                    nc.scalar.copy(dst[:, 512 * n:512 * (n + 1)], acc[:])

            def emit_v():
                for t in range(NKT):
                    acc = ps.tile([128, 256], F32, tag="ps", name=f"vacc_{t}")
                    for k in range(8):
                        nc.tensor.matmul(
                            acc[:],
                            lhsT=xT_sb[:, S * k + 128 * t: S * k + 128 * (t + 1)],
                            rhs=wv_sb[:, 256 * k:256 * (k + 1)],
                            start=(k == 0), stop=False)
                    nc.tensor.matmul(acc[:], lhsT=ones_sb[0:1, 0:128],
                                     rhs=bv_sb[0:1, :], start=False, stop=True)
                    dst = v_sb[:, 260 * t:260 * (t + 1)].rearrange(
                        "p (h c) -> p h c", c=65)[:, :, 0:64]
                    nc.scalar.copy(
                        dst, acc[:].rearrange("p (h c) -> p h c", c=64))

            # ---- attention ----------------------------------------------
            def emit_attn(p):
                qt = qk_sb[f"q{p}"]
                kt = qk_sb[f"k{p}"]
                for qc in range(NQC):
                    q0 = QC * qc
                    ctx_ps = [ctxp.tile([65, QC], F32, tag="ctx",
                                        name=f"ctx_{p}_{qc}_{hl}")
                              for hl in range(2)]
                    ntiles = 4 * qc + 4
                    e_tiles = [None] * ntiles
                    cols = [None] * ntiles

                    def emit_av(t):
                        col0 = cols[t]
                        for hl in range(2):
                            h4 = 2 * p + hl
                            nc.tensor.matmul(
                                ctx_ps[hl][:, col0:QC],
                                lhsT=v_sb[:, 260 * t + 65 * h4: 260 * t + 65 * h4 + 65],
                                rhs=e_tiles[t][:, QC * hl:QC * hl + QC - col0],
                                start=(t == 0), stop=(t == ntiles - 1),
                                skip_group_check=True)

                    for t in range(ntiles):
                        col0 = max(0, 128 * t - q0)
                        cols[t] = col0
                        neff = QC - col0
                        s_ps = sp2.tile([128, 2 * QC], F32, tag="sps")
                        e_sb = wp.tile([128, 2 * QC], BF16, tag="e")
                        for hl in range(2):
                            nc.tensor.matmul(
                                s_ps[:, QC * hl:QC * hl + neff],
                                lhsT=kt[64 * hl:64 * (hl + 1), 128 * t:128 * (t + 1)],
                                rhs=qt[64 * hl:64 * (hl + 1), q0 + col0:q0 + QC],
                                start=True, stop=True)
                        sv = s_ps.rearrange("p (h q) -> p h q", h=2)[:, :, 0:neff]
                        ev = e_sb.rearrange("p (h q) -> p h q", h=2)[:, :, 0:neff]
                        nc.scalar.activation(ev, sv, EXP, scale=SCALE)
                        if t >= 4 * qc:
                            for hl in range(2):
                                nc.vector.tensor_mul(
                                    e_sb[:, QC * hl:QC * hl + 128],
                                    e_sb[:, QC * hl:QC * hl + 128], tri_sb[:])
                        e_tiles[t] = e_sb
                        if t > 0:
                            emit_av(t - 1)
                    emit_av(ntiles - 1)

                    # normalize + stage for the collective
                    ctxn = np2.tile([128, QC], BF16, tag="ctxn")
                    for hl in range(2):
                        recip = np2.tile([1, QC], F32, tag="recip")
                        nc.vector.reciprocal(recip[:], ctx_ps[hl][64:65, :])
                        bc_sb = np2.tile([64, QC], F32, tag="bc")
                        nc.gpsimd.partition_broadcast(bc_sb[:], recip[:])
                        nc.vector.tensor_mul(
                            ctxn[64 * hl:64 * (hl + 1), :],
                            ctx_ps[hl][0:64, :], bc_sb[:])
                    # shard slots qc and qc+4 (same data for both batch groups)
                    nc.sync.dma_start(
                        cc_in[p][128 * qc:128 * (qc + 1), :], ctxn[:])
                    nc.sync.dma_start(
                        cc_in[p][512 + 128 * qc:512 + 128 * (qc + 1), :], ctxn[:])

            def emit_a2a(p):
                nc.gpsimd.collective_compute(
                    "AllToAll", mybir.AluOpType.bypass,
                    replica_groups=[[0, 1, 2, 3, 4, 5, 6, 7]],
                    ins=[cc_in[p].ap().opt()], outs=[cc_out[p].ap().opt()])

            co_sb = [None, None]

            def emit_co_load(p):
                co_sb[p] = cp.tile([128, 8 * QC], BF16, tag=f"co{p}",
                                   name=f"co_{p}")
                if do_cc:
                    nc.sync.dma_start(
                        co_sb[p][:],
                        cc_out[p].ap().rearrange("(n p) m -> p n m", p=128))
                else:
                    nc.vector.memset(co_sb[p][:, 0:512], 0.0)

            part_sb = [None]

            def emit_proj_half(p, out_sb):
                # out^T [o, s] contribution of pair p's 8 rank-chunks
                if p == 0:
                    part_sb[0] = cp.tile([128, 8 * QC], F32, tag="part",
                                         name="part_sb")
                for ot in range(8):
                    acc = ps.tile([128, QC], F32, tag="ps",
                                  name=f"oacc_{p}_{ot}")
                    for i in range(8):
                        c16 = 8 * p + i
                        nc.tensor.matmul(
                            acc[:],
                            lhsT=wout_sb[:, E * c16 + 128 * ot: E * c16 + 128 * (ot + 1)],
                            rhs=co_sb[p][:, QC * i:QC * (i + 1)],
                            start=(i == 0), stop=(p == 0 and i == 7))
                    if p == 1:
                        nc.tensor.matmul(
                            acc[:], lhsT=bout_sb[0:1, 128 * ot:128 * (ot + 1)],
                            rhs=ones_sb[0:1, 0:QC], start=False, stop=True)
                    if p == 0:
                        nc.scalar.copy(
                            part_sb[0][:, QC * ot:QC * (ot + 1)], acc[:])
                    else:
                        nc.vector.tensor_add(
                            out_sb[:, QC * ot:QC * (ot + 1)],
                            part_sb[0][:, QC * ot:QC * (ot + 1)], acc[:])

            out_sb = cp.tile([128, 8 * QC], F32, tag="osb")
            if do_qkv:
                emit_qk("q0", 0)
                emit_qk("k0", 2)
                emit_v()
            if do_attn:
                emit_attn(0)
            if do_cc:
                emit_a2a(0)
            if do_qkv:
                emit_qk("q1", 1)
                emit_qk("k1", 3)
            if do_proj:
                emit_co_load(0)
                emit_proj_half(0, out_sb)
            if do_attn:
                emit_attn(1)
            if do_cc:
                emit_a2a(1)
            if do_proj:
                emit_co_load(1)
                emit_proj_half(1, out_sb)
            else:
                nc.vector.memset(out_sb[:], 0.0)
            nc.sync.dma_start(
                out.ap().rearrange("(t p) m -> p t m", p=128), out_sb[:])

    nc.compile()
    return nc


def make_in_maps(inputs, w_qkv, b_qkv, w_out, b_out):
    bf = ml_dtypes.bfloat16
    xT = [np.ascontiguousarray(inputs[b].T).astype(bf) for b in range(B)]
    in_maps = []
    for c in range(N_CORES):
        b = c // 4
        hg = c % 4
        rows = slice(256 * hg, 256 * (hg + 1))
        w_q = w_qkv[0 * E:1 * E][rows]          # [256, 1024]
        w_k = w_qkv[1 * E:2 * E][rows]
        w_v = w_qkv[2 * E:3 * E][rows]
        wqkT = np.ascontiguousarray(
            np.concatenate([w_q, w_k], axis=0).T).astype(bf)   # [1024, 512]
        wvT = np.ascontiguousarray(w_v.T).astype(bf)           # [1024, 256]
        # stacked + permuted w_out^T: chunk (p, i) rows map to rank i's
        # pair-p heads {4i+2p, 4i+2p+1}; zero for ranks outside this group
        wo = np.zeros((2, 8, 128, E), np.float32)
        for p in range(2):
            for i in range(4 * b, 4 * b + 4):
                e0 = 64 * (4 * (i % 4) + 2 * p)
                wo[p, i] = w_out[:, e0:e0 + 128].T
        woutT = wo.reshape(16 * 128, E).astype(bf)             # [2048, 1024]
        bqk = np.concatenate(
            [b_qkv[0 * E:1 * E][rows], b_qkv[1 * E:2 * E][rows]]
        ).reshape(1, 512).astype(bf)
        bvv = b_qkv[2 * E:3 * E][rows].reshape(1, 256).astype(bf)
        bo = b_out.reshape(1, E).astype(bf)
        in_maps.append({
            "xT": xT[b], "wqkT": wqkT, "wvT": wvT, "woutT": woutT,
            "bqk": bqk, "bv": bvv, "bout": bo,
        })
    return in_maps


def assemble(results):
    out = np.empty((B, S, E), np.float32)
    for c in range(N_CORES):
        b, hg = c // 4, c % 4
        out[b, 512 * hg:512 * (hg + 1), :] = results[c]["out"].T
    return out


_cached_nc = None


def kernel(inputs, w_qkv, b_qkv, w_out, b_out):
    global _cached_nc
    from concourse.bass_utils import run_bass_kernel_spmd
    if _cached_nc is None:
        _cached_nc = build_nc()
    in_maps = make_in_maps(inputs, w_qkv, b_qkv, w_out, b_out)
    res = run_bass_kernel_spmd(
        _cached_nc, in_maps, core_ids=list(range(N_CORES)), trace=False)
    return assemble(res.results)


# revision 12
# speedup vs baseline: 18.2753x; 18.2753x over previous
"""Trainium2 Bass kernel for nn_MultiHeadAttention (B=2, S=2048, E=1024, H=16).

Sharding (8 cores): core c handles batch b = c//4 and the 4 heads
[4*(c%4), 4*(c%4)+4) of that batch. Per core:
  1. QKV projection from the (transposed, bf16) input slab:
     Q^T, K^T in [d, s] layout (one head-pair per 128-partition tile),
     V in natural [s, d] layout with an extra all-ones column per head
     (the ones column makes the A@V matmul also produce the softmax
     denominator row).
  2. Flash-style causal attention per head pair over 512-query chunks:
     scores computed transposed S^T[k, q] one 128-key tile at a time
     (both heads of a pair into one 2-bank PSUM tile), exp on the Scalar
     engine (no max-subtraction: |scores| <= ~3 for these inputs), causal
     masking via an upper-triangular multiplicative mask on the diagonal
     block, A@V accumulated in PSUM; normalization uses a DVE reciprocal
     of the denominator row and a rank-1 PE matmul to broadcast it across
     partitions.
  3. Per head-pair, ctx^T is exchanged with one 8-core AllToAll: each core
     sends its pair ctx for s-quarter j to shard slots j and j+4, so every
     core receives the full-head ctx^T for its own s-quarter; rows from the
     other batch's cores are killed by zeros in the host-permuted stacked
     w_out. The pair-0 AllToAll overlaps pair-1 compute.
  4. Output projection (full E contraction over the 16 received chunks)
     produces out^T [1024, 512] for this core's s-quarter.
Host gathers the 8 [1024, 512] fp32 slabs into the [2, 2048, 1024] output.

All matmuls run in bf16 (fp32 PSUM accumulation); softmax statistics stay
fp32 except the broadcast reciprocal row (bf16).
"""
import sys

if '/opt/trn_rl_repo' not in sys.path:
    sys.path.insert(0, '/opt/trn_rl_repo')

from contextlib import ExitStack

import numpy as np
import ml_dtypes

import concourse.bass as bass
import concourse.bacc as bacc
import concourse.tile as tile
from concourse import mybir

BF16 = mybir.dt.bfloat16
F32 = mybir.dt.float32
EXP = mybir.ActivationFunctionType.Exp

B, S, E = 2, 2048, 1024
H, D = 16, 64
HPC = 4              # heads per core
N_CORES = 8
QC = 512             # query chunk
NQC = S // QC        # 4
NKT = S // 128       # 16 key tiles
SCALE = 1.0 / np.sqrt(D)


def build_nc(do_qkv=True, do_attn=True, do_cc=True, do_proj=True, reps=1):
    nc = bacc.Bacc("TRN2", target_bir_lowering=False, debug=False,
                   num_devices=N_CORES)

    xT = nc.dram_tensor("xT", [E, S], BF16, kind="ExternalInput")
    wqkT = nc.dram_tensor("wqkT", [E, 512], BF16, kind="ExternalInput")
    wvT = nc.dram_tensor("wvT", [E, 256], BF16, kind="ExternalInput")
    woutT = nc.dram_tensor("woutT", [2 * E, E], BF16, kind="ExternalInput")
    bqk = nc.dram_tensor("bqk", [1, 512], BF16, kind="ExternalInput")
    bv = nc.dram_tensor("bv", [1, 256], BF16, kind="ExternalInput")
    bout = nc.dram_tensor("bout", [1, E], BF16, kind="ExternalInput")
    out = nc.dram_tensor("out", [E, QC], F32, kind="ExternalOutput")

    cc_in = [nc.dram_tensor(f"cc_in{p}", [1024, QC], BF16) for p in range(2)]
    cc_out = [nc.dram_tensor(f"cc_out{p}", [1024, QC], BF16)
              for p in range(2)]

    tri_np = np.triu(np.ones((128, 128), np.float32)).astype(ml_dtypes.bfloat16)
    tri_dram = nc.inline_tensor(tri_np, name="tri_const")
    ones_np = np.ones((1, 512), ml_dtypes.bfloat16)
    ones_dram = nc.inline_tensor(ones_np, name="ones_const")

    with tile.TileContext(nc) as tc, ExitStack() as ctx:
        cp = ctx.enter_context(tc.tile_pool(name="const", bufs=1))
        wp = ctx.enter_context(tc.tile_pool(name="work", bufs=8))
        np2 = ctx.enter_context(tc.tile_pool(name="norm", bufs=3))
        ps = ctx.enter_context(tc.tile_pool(name="ps", bufs=2, space="PSUM"))
        sp2 = ctx.enter_context(tc.tile_pool(name="sp2", bufs=2, space="PSUM"))
        ctxp = ctx.enter_context(tc.tile_pool(name="ctxp", bufs=2, space="PSUM"))

        # ---- constant / input loads -------------------------------------
        xT_sb = cp.tile([128, 8 * S], BF16, tag="xT")
        nc.sync.dma_start(xT_sb[:], xT.ap().rearrange("(n p) m -> p n m", p=128))
        wqk_sb = cp.tile([128, 8 * 512], BF16, tag="wqk")
        nc.sync.dma_start(wqk_sb[:], wqkT.ap().rearrange("(n p) m -> p n m", p=128))
        wv_sb = cp.tile([128, 8 * 256], BF16, tag="wv")
        nc.sync.dma_start(wv_sb[:], wvT.ap().rearrange("(n p) m -> p n m", p=128))
        wout_sb = cp.tile([128, 16 * E], BF16, tag="wout")
        nc.sync.dma_start(wout_sb[:], woutT.ap().rearrange("(n p) m -> p n m", p=128))
        bqk_sb = cp.tile([1, 512], BF16, tag="bqk")
        nc.sync.dma_start(bqk_sb[:], bqk.ap())
        bv_sb = cp.tile([1, 256], BF16, tag="bv")
        nc.sync.dma_start(bv_sb[:], bv.ap())
        bout_sb = cp.tile([1, E], BF16, tag="bout")
        nc.sync.dma_start(bout_sb[:], bout.ap())
        tri_sb = cp.tile([128, 128], BF16, tag="tri")
        nc.sync.dma_start(tri_sb[:], tri_dram.ap())
        ones_sb = cp.tile([1, 512], BF16, tag="ones")
        nc.sync.dma_start(ones_sb[:], ones_dram.ap())

        # V slab: 16 s-tiles x 4 heads x (64 V cols + 1 ones col)
        v_sb = cp.tile([128, NKT * 260], BF16, tag="v")
        for t in range(NKT):
            blk = v_sb[:, 260 * t:260 * (t + 1)].rearrange(
                "p (h c) -> p h c", c=65)
            nc.vector.memset(blk[:, :, 64:65], 1.0)

        for _rep in range(reps):
            # ---- QKV projection helpers ---------------------------------
            qk_sb = {}

            def emit_qk(name, m):
                dst = cp.tile([128, S], BF16, tag=f"qk_{name}", name=f"qk_{name}")
                qk_sb[name] = dst
                for n in range(4):
                    acc = ps.tile([128, 512], F32, tag="ps", name=f"qkacc_{name}_{n}")
                    for k in range(8):
                        nc.tensor.matmul(
                            acc[:],
                            lhsT=wqk_sb[:, 512 * k + 128 * m: 512 * k + 128 * (m + 1)],
                            rhs=xT_sb[:, S * k + 512 * n: S * k + 512 * (n + 1)],
                            start=(k == 0), stop=False)
                    nc.tensor.matmul(
                        acc[:], lhsT=bqk_sb[0:1, 128 * m:128 * (m + 1)],
                        rhs=ones_sb[0:1, 0:512], start=False, stop=True)
                    nc.scalar.copy(dst[:, 512 * n:512 * (n + 1)], acc[:])

            def emit_v():
                for t in range(NKT):
                    acc = ps.tile([128, 256], F32, tag="ps", name=f"vacc_{t}")
                    for k in range(8):
                        nc.tensor.matmul(
                            acc[:],
                            lhsT=xT_sb[:, S * k + 128 * t: S * k + 128 * (t + 1)],
                            rhs=wv_sb[:, 256 * k:256 * (k + 1)],
                            start=(k == 0), stop=False)
                    nc.tensor.matmul(acc[:], lhsT=ones_sb[0:1, 0:128],
                                     rhs=bv_sb[0:1, :], start=False, stop=True)
                    dst = v_sb[:, 260 * t:260 * (t + 1)].rearrange(
                        "p (h c) -> p h c", c=65)[:, :, 0:64]
                    nc.scalar.copy(
                        dst, acc[:].rearrange("p (h c) -> p h c", c=64))

            # ---- attention ----------------------------------------------
            def emit_attn(p):
                qt = qk_sb[f"q{p}"]
                kt = qk_sb[f"k{p}"]
                for qc in range(NQC):
                    q0 = QC * qc
                    ctx_ps = [ctxp.tile([65, QC], F32, tag="ctx",
                                        name=f"ctx_{p}_{qc}_{hl}")
                              for hl in range(2)]
                    ntiles = 4 * qc + 4
                    e_tiles = [None] * ntiles
                    cols = [None] * ntiles

                    def emit_av(t):
                        col0 = cols[t]
                        for hl in range(2):
                            h4 = 2 * p + hl
                            nc.tensor.matmul(
                                ctx_ps[hl][:, col0:QC],
                                lhsT=v_sb[:, 260 * t + 65 * h4: 260 * t + 65 * h4 + 65],
                                rhs=e_tiles[t][:, QC * hl:QC * hl + QC - col0],
                                start=(t == 0), stop=(t == ntiles - 1),
                                skip_group_check=True)

                    for t in range(ntiles):
                        col0 = max(0, 128 * t - q0)
                        cols[t] = col0
                        neff = QC - col0
                        s_ps = sp2.tile([128, 2 * QC], F32, tag="sps")
                        e_sb = wp.tile([128, 2 * QC], BF16, tag="e")
                        for hl in range(2):
                            nc.tensor.matmul(
                                s_ps[:, QC * hl:QC * hl + neff],
                                lhsT=kt[64 * hl:64 * (hl + 1), 128 * t:128 * (t + 1)],
                                rhs=qt[64 * hl:64 * (hl + 1), q0 + col0:q0 + QC],
                                start=True, stop=True)
                        sv = s_ps.rearrange("p (h q) -> p h q", h=2)[:, :, 0:neff]
                        ev = e_sb.rearrange("p (h q) -> p h q", h=2)[:, :, 0:neff]
                        nc.scalar.activation(ev, sv, EXP, scale=SCALE)
                        if t >= 4 * qc:
                            for hl in range(2):
                                nc.vector.tensor_mul(
                                    e_sb[:, QC * hl:QC * hl + 128],
                                    e_sb[:, QC * hl:QC * hl + 128], tri_sb[:])
                        e_tiles[t] = e_sb
                        if t > 0:
                            emit_av(t - 1)
                    emit_av(ntiles - 1)

                    # normalize + stage for the collective
                    ctxn = np2.tile([128, QC], BF16, tag="ctxn")
                    for hl in range(2):
                        recip = np2.tile([1, QC], F32, tag="recip")
                        nc.vector.reciprocal(recip[:], ctx_ps[hl][64:65, :])
                        bc_sb = np2.tile([64, QC], F32, tag="bc")
                        nc.gpsimd.partition_broadcast(bc_sb[:], recip[:])
                        nc.vector.tensor_mul(
                            ctxn[64 * hl:64 * (hl + 1), :],
                            ctx_ps[hl][0:64, :], bc_sb[:])
                    # shard slots qc and qc+4 (same data for both batch groups)
                    nc.sync.dma_start(
                        cc_in[p][128 * qc:128 * (qc + 1), :], ctxn[:])
                    nc.sync.dma_start(
                        cc_in[p][512 + 128 * qc:512 + 128 * (qc + 1), :], ctxn[:])

            def emit_a2a(p):
                nc.gpsimd.collective_compute(
                    "AllToAll", mybir.AluOpType.bypass,
                    replica_groups=[[0, 1, 2, 3, 4, 5, 6, 7]],
                    ins=[cc_in[p].ap().opt()], outs=[cc_out[p].ap().opt()])

            co_sb = [None, None]

            def emit_co_load(p):
                co_sb[p] = cp.tile([128, 8 * QC], BF16, tag=f"co{p}",
                                   name=f"co_{p}")
                if do_cc:
                    nc.sync.dma_start(
                        co_sb[p][:],
                        cc_out[p].ap().rearrange("(n p) m -> p n m", p=128))
                else:
                    nc.vector.memset(co_sb[p][:, 0:512], 0.0)

            part_sb = [None]

            def emit_proj_half(p, out_sb):
                # out^T [o, s] contribution of pair p's 8 rank-chunks
                if p == 0:
                    part_sb[0] = cp.tile([128, 8 * QC], F32, tag="part",
                                         name="part_sb")
                for ot in range(8):
                    acc = ps.tile([128, QC], F32, tag="ps",
                                  name=f"oacc_{p}_{ot}")
                    for i in range(8):
                        c16 = 8 * p + i
                        nc.tensor.matmul(
                            acc[:],
                            lhsT=wout_sb[:, E * c16 + 128 * ot: E * c16 + 128 * (ot + 1)],
                            rhs=co_sb[p][:, QC * i:QC * (i + 1)],
                            start=(i == 0), stop=(p == 0 and i == 7))
                    if p == 1:
                        nc.tensor.matmul(
                            acc[:], lhsT=bout_sb[0:1, 128 * ot:128 * (ot + 1)],
                            rhs=ones_sb[0:1, 0:QC], start=False, stop=True)
                    if p == 0:
                        nc.scalar.copy(
                            part_sb[0][:, QC * ot:QC * (ot + 1)], acc[:])
                    else:
                        nc.vector.tensor_add(
                            out_sb[:, QC * ot:QC * (ot + 1)],
                            part_sb[0][:, QC * ot:QC * (ot + 1)], acc[:])

            out_sb = cp.tile([128, 8 * QC], F32, tag="osb")
            if do_qkv:
                emit_qk("q0", 0)
                emit_qk("k0", 2)
                emit_v()
            if do_attn:
                emit_attn(0)
            if do_cc:
                emit_a2a(0)
            if do_qkv:
                emit_qk("q1", 1)
                emit_qk("k1", 3)
            if do_proj:
                emit_co_load(0)
                emit_proj_half(0, out_sb)
            if do_attn:
                emit_attn(1)
            if do_cc:
                emit_a2a(1)
            if do_proj:
                emit_co_load(1)
                emit_proj_half(1, out_sb)
            else:
                nc.vector.memset(out_sb[:], 0.0)
            nc.sync.dma_start(
                out.ap().rearrange("(t p) m -> p t m", p=128), out_sb[:])

    nc.compile()
    return nc


def make_in_maps(inputs, w_qkv, b_qkv, w_out, b_out):
    bf = ml_dtypes.bfloat16
    xT = [np.ascontiguousarray(inputs[b].T).astype(bf) for b in range(B)]
    in_maps = []
    for c in range(N_CORES):
        b = c // 4
        hg = c % 4
        rows = slice(256 * hg, 256 * (hg + 1))
        w_q = w_qkv[0 * E:1 * E][rows]          # [256, 1024]
        w_k = w_qkv[1 * E:2 * E][rows]
        w_v = w_qkv[2 * E:3 * E][rows]
        wqkT = np.ascontiguousarray(
            np.concatenate([w_q, w_k], axis=0).T).astype(bf)   # [1024, 512]
        wvT = np.ascontiguousarray(w_v.T).astype(bf)           # [1024, 256]
        # stacked + permuted w_out^T: chunk (p, i) rows map to rank i's
        # pair-p heads {4i+2p, 4i+2p+1}; zero for ranks outside this group
        wo = np.zeros((2, 8, 128, E), np.float32)
        for p in range(2):
            for i in range(4 * b, 4 * b + 4):
                e0 = 64 * (4 * (i % 4) + 2 * p)
                wo[p, i] = w_out[:, e0:e0 + 128].T
        woutT = wo.reshape(16 * 128, E).astype(bf)             # [2048, 1024]
        bqk = np.concatenate(
            [b_qkv[0 * E:1 * E][rows], b_qkv[1 * E:2 * E][rows]]
        ).reshape(1, 512).astype(bf)
        bvv = b_qkv[2 * E:3 * E][rows].reshape(1, 256).astype(bf)
        bo = b_out.reshape(1, E).astype(bf)
        in_maps.append({
            "xT": xT[b], "wqkT": wqkT, "wvT": wvT, "woutT": woutT,
            "bqk": bqk, "bv": bvv, "bout": bo,
        })
    return in_maps


def assemble(results):
    out = np.empty((B, S, E), np.float32)
    for c in range(N_CORES):
        b, hg = c // 4, c % 4
        out[b, 512 * hg:512 * (hg + 1), :] = results[c]["out"].T
    return out


_cached_nc = None


def kernel(inputs, w_qkv, b_qkv, w_out, b_out):
    global _cached_nc
    from concourse.bass_utils import run_bass_kernel_spmd
    if _cached_nc is None:
        _cached_nc = build_nc()
    in_maps = make_in_maps(inputs, w_qkv, b_qkv, w_out, b_out)
    res = run_bass_kernel_spmd(
        _cached_nc, in_maps, core_ids=list(range(N_CORES)), trace=False)
    return assemble(res.results)
